# revision 16
# baseline (speedup 1.0000x reference)
"""Trainium2 Bass kernel: causal multi-head attention with interleaved RoPE.

Problem shapes (hardcoded): x [2, 2048, 1024], 16 heads of dk=64.
Sharding: 8 cores = 2 batches x 4 head-groups (4 heads each). Each core
computes its head-slice Q/K/V projections, RoPE, causal attention, and a
partial output through its Wo row-slice; the host sums the 4 partials per
batch and adds bo.

Numerics: bf16 operands everywhere (validated ~5e-3 rel err vs the 2e-2
gate), fp32 PSUM accumulation.

Schedule: one flat instruction stream interleaving the three phases so the
PE never idles: projection units of block j+1 and Wo units of block j-1 are
issued as fill between attention score/PV groups of block j (PV lags its
score group by one slot to hide the exp latency). Causal masking accumulates
a -200 mask into the score psum via identity-stationary matmuls; for j>0
the score/exp/PV work on diagonal groups is trimmed to the columns right of
each key tile (the fully-masked left columns are skipped outright, so only
the 128-wide diagonal triangle needs masking). Softmax normalization runs
entirely off the PE: denominator row staged to SBUF (custom-DVE ops cannot
read PSUM on hw), reciprocal_approx_fast (DVE) -> partition_broadcast
(gpsimd) -> mul (DVE). Wo packs head pairs on the contraction dim (128 rows
fully used). DMA: few large blocked transfers split across both hwdge
queues (sync + Activation) so the first projection starts ~11us in; outputs
stream out per 128-row block as Wo psums complete.

RoPE trick: attention scores are invariant to any permutation of the dk
axis applied to both Q and K, so the Wq/Wk columns are permuted on the host
into a "quadrant half-split" layout where each rotation pair partner sits
exactly 16 partitions away inside the same 32-partition quadrant. The DVE
stream_shuffle (a per-quadrant 32-way permute) then produces the swapped
operand, and RoPE becomes: rot = q * cosT + shuffle(q) * sinT with
host-precomputed tables (sinT carries the sign).
"""

from collections import deque
from contextlib import ExitStack

import numpy as np
import ml_dtypes

import concourse.bass as bass
import concourse.mybir as mybir
import concourse.tile as tile

B, S, D, H = 2, 2048, 1024, 16
DK = D // H  # 64
HG = 4  # heads per core
NCOLS = HG * DK  # 256 columns of the projection per core
THETA = 10000.0
SCALE = 1.0 / float(np.sqrt(DK))
N_CORES = 8

SB = 512            # sq block width
NSB = S // SB       # 4
NST = S // 128      # 16 key tiles / V tiles
NDC = D // 128      # 8 contraction chunks
GW = 2              # key tiles per score-psum group
AUGW = 72           # V head stride (65 used), 72*2B = 144 = 9*16B aligned

F32 = mybir.dt.float32
F32R = mybir.dt.float32r
BF16 = mybir.dt.bfloat16
BF = ml_dtypes.bfloat16
MASKVAL = -200.0


# ---------------------------------------------------------------------------
# host-side prep
# ---------------------------------------------------------------------------

def _rope_perm():
    """Within-head column permutation pi: new row r -> original dk index."""
    perm = np.empty(DK, dtype=np.int64)
    for r in range(DK):
        q, m = divmod(r, 32)
        if m < 16:
            perm[r] = 2 * (16 * q + m)
        else:
            perm[r] = 2 * (16 * q + m - 16) + 1
    return perm


_PERM = _rope_perm()
SHUF_MASK = list(range(16, 32)) + list(range(16))  # swap 16-halves per quadrant


def _rope_tables(pos):
    """cosT/sinT [128, S] fp32 for the permuted layout. pos: [S] int."""
    inv_freq = (np.float32(THETA) ** (-(np.arange(0, DK, 2, dtype=np.float32) / np.float32(DK))))  # [32]
    ang = pos.astype(np.float32)[:, None] * inv_freq[None, :]  # [S, 32]
    cos = np.cos(ang)
    sin = np.sin(ang)
    cosT = np.empty((128, S), dtype=np.float32)
    sinT = np.empty((128, S), dtype=np.float32)
    for p in range(128):
        r = p % DK
        q, m = divmod(r, 32)
        if m < 16:
            i = 16 * q + m
            sgn = -1.0
        else:
            i = 16 * q + m - 16
            sgn = 1.0
        cosT[p] = cos[:, i]
        sinT[p] = np.float32(sgn) * sin[:, i]
    return cosT, sinT


def bf16(a):
    return np.ascontiguousarray(np.asarray(a, dtype=np.float32).astype(BF))


def make_core_inputs(x, token_position, Wq, bq, Wk, bk, Wv, bv, Wo, bo):
    """Build the 8 per-core input maps."""
    x = np.asarray(x, dtype=np.float32)
    token_position = np.asarray(token_position)
    Wq, Wk, Wv, Wo = (np.asarray(w, dtype=np.float32) for w in (Wq, Wk, Wv, Wo))
    bq, bk, bv = (np.asarray(b_, dtype=np.float32) for b_ in (bq, bk, bv))

    # mask slabs for the 4 diagonal key-tile offsets d: key 128d+p masks
    # query q (tile-relative) iff 128d+p > q; -200 added into the score psum
    pp = np.arange(128)[:, None]
    qq = np.arange(SB)[None, :]
    maskslab = bf16(np.stack(
        [np.where(128 * dd + pp > qq, np.float32(MASKVAL), np.float32(0.0))
         for dd in range(4)], axis=1))  # [128, 4, 512]
    ident = bf16(np.eye(128, dtype=np.float32))
    onesr = np.ones((1, DK), dtype=np.float32)

    in_maps = []
    tables = {}
    for c in range(N_CORES):
        b, hg = divmod(c, HG)
        heads = range(HG * hg, HG * hg + HG)
        cols_qk = np.concatenate([DK * h + _PERM for h in heads])
        cols_v = np.arange(NCOLS * hg, NCOLS * hg + NCOLS)
        if b not in tables:
            tables[b] = _rope_tables(np.asarray(token_position[b]))
        cosT, sinT = tables[b]
        # wo rows packed as head pairs: [:, cpair, :] rows 0-63 head 2c, 64-127 head 2c+1
        wo_dev = np.stack([Wo[cols_v[128 * cp:128 * cp + 128], :] for cp in range(2)],
                          axis=1)  # [128, 2, 1024]
        def blk_w(w):  # [1024, 256] -> [128, 8, 256] (dc-blocked, partition-major)
            return bf16(np.ascontiguousarray(w.reshape(NDC, 128, NCOLS).transpose(1, 0, 2)))
        xT_blk = bf16(np.ascontiguousarray(
            x[b].T.reshape(NDC, 128, NSB, SB).transpose(2, 1, 0, 3)))  # [4, 128, 8, 512]
        in_maps.append({
            "xT": xT_blk,                             # [4, 128, 8, 512]
            "wq": blk_w(Wq[:, cols_qk]),              # [128, 8, 256]
            "wk": blk_w(Wk[:, cols_qk]),
            "wv": blk_w(Wv[:, cols_v]),
            "wo": bf16(wo_dev),                       # [128, 2, 1024]
            "bq": bf16(bq[cols_qk][None, :]),         # [1, 256]
            "bk": bf16(bk[cols_qk][None, :]),
            "bv": bf16(bv[cols_v][None, :]),
            "ones_row": bf16(np.ones((1, SB), np.float32)),
            "onesr": onesr,                           # [1, 64] fp32 (f32r param)
            "ident": ident,                           # [128, 128]
            "maskslab": maskslab,                     # [128, 4, 512]
            "cosT": bf16(cosT),
            "sinT": bf16(sinT),
        })
    return in_maps


# ---------------------------------------------------------------------------
# device program
# ---------------------------------------------------------------------------

def build_program(with_bias=False):
    from concourse import bacc, library_config
    nc = bacc.Bacc("TRN2", debug=False)

    xT = nc.declare_dram_parameter("xT", [NSB, 128, NDC, SB], BF16, isOutput=False).ap()
    wq = nc.declare_dram_parameter("wq", [128, NDC, NCOLS], BF16, isOutput=False).ap()
    wk = nc.declare_dram_parameter("wk", [128, NDC, NCOLS], BF16, isOutput=False).ap()
    wv = nc.declare_dram_parameter("wv", [128, NDC, NCOLS], BF16, isOutput=False).ap()
    wo = nc.declare_dram_parameter("wo", [128, 2, D], BF16, isOutput=False).ap()
    bq = nc.declare_dram_parameter("bq", [1, NCOLS], BF16, isOutput=False).ap()
    bk = nc.declare_dram_parameter("bk", [1, NCOLS], BF16, isOutput=False).ap()
    bv = nc.declare_dram_parameter("bv", [1, NCOLS], BF16, isOutput=False).ap()
    ones_row_d = nc.declare_dram_parameter("ones_row", [1, SB], BF16, isOutput=False).ap()
    onesr_d = nc.declare_dram_parameter("onesr", [1, DK], F32R, isOutput=False).ap()
    ident_d = nc.declare_dram_parameter("ident", [128, 128], BF16, isOutput=False).ap()
    maskslab_d = nc.declare_dram_parameter("maskslab", [128, 4, SB], BF16, isOutput=False).ap()
    cosT = nc.declare_dram_parameter("cosT", [128, S], BF16, isOutput=False).ap()
    sinT = nc.declare_dram_parameter("sinT", [128, S], BF16, isOutput=False).ap()
    out = nc.declare_dram_parameter("out", [S, D], BF16, isOutput=True).ap()

    with tile.TileContext(nc) as tc, ExitStack() as ctx:
        nc.gpsimd.load_library(library_config.proxy)
        const = ctx.enter_context(tc.tile_pool(name="const", bufs=1))
        sbig = ctx.enter_context(tc.tile_pool(name="sbig", bufs=1))
        xts = ctx.enter_context(tc.tile_pool(name="xts", bufs=2))
        rtmp = ctx.enter_context(tc.tile_pool(name="rtmp", bufs=2))
        epool = ctx.enter_context(tc.tile_pool(name="epool", bufs=3))
        npool = ctx.enter_context(tc.tile_pool(name="npool", bufs=2))
        bpool = ctx.enter_context(tc.tile_pool(name="bpool", bufs=2))
        opool = ctx.enter_context(tc.tile_pool(name="opool", bufs=2))
        pj_ps = ctx.enter_context(tc.tile_pool(name="pj_ps", bufs=2, space="PSUM"))
        sc_ps = ctx.enter_context(tc.tile_pool(name="sc_ps", bufs=2, space="PSUM"))
        pv_ps = ctx.enter_context(tc.tile_pool(name="pv_ps", bufs=2, space="PSUM"))

        # --- DMA prelude: few large transfers, first projection's operands first
        wq_sb = const.tile([128, NDC, NCOLS], BF16, tag="wq")
        wk_sb = const.tile([128, NDC, NCOLS], BF16, tag="wk")
        wv_sb = const.tile([128, NDC, NCOLS], BF16, tag="wv")
        xt_tiles = {}

        def prefetch_xt(sb):
            t = xts.tile([128, NDC, SB], BF16, tag="xt", name=f"xt{sb}")
            nc.sync.dma_start(t[:], xT[sb])
            xt_tiles[sb] = t

        # halve the first transfers so the first projection matmuls start
        # sooner; wk rides the second (Activation) hwdge queue meanwhile
        xt0 = xts.tile([128, NDC, SB], BF16, tag="xt", name="xt0")
        nc.sync.dma_start(wq_sb[:, 0:4, :], wq[:, 0:4, :])
        nc.sync.dma_start(xt0[:, 0:4, :], xT[0, :, 0:4, :])
        cos_sb = const.tile([128, S], BF16, tag="cos")
        sin_sb = const.tile([128, S], BF16, tag="sin")
        nc.scalar.dma_start(wk_sb[:], wk)
        nc.scalar.dma_start(cos_sb[:, 0:SB], cosT[:, 0:SB])
        nc.scalar.dma_start(sin_sb[:, 0:SB], sinT[:, 0:SB])
        nc.sync.dma_start(wq_sb[:, 4:, :], wq[:, 4:, :])
        nc.sync.dma_start(xt0[:, 4:, :], xT[0, :, 4:, :])
        xt_tiles[0] = xt0
        ident_sb = const.tile([128, 128], BF16, tag="ident")
        nc.scalar.dma_start(ident_sb[:], ident_d)
        maskslab_sb = const.tile([128, 4, SB], BF16, tag="maskslab")
        nc.scalar.dma_start(maskslab_sb[:], maskslab_d)
        nc.scalar.dma_start(wv_sb[:], wv)
        nc.scalar.dma_start(cos_sb[:, SB:], cosT[:, SB:])
        nc.scalar.dma_start(sin_sb[:, SB:], sinT[:, SB:])
        onesr_sb = const.tile([1, DK], F32R, tag="onesr")
        nc.sync.dma_start(onesr_sb[:], onesr_d)
        if with_bias:
            bq_sb = const.tile([1, NCOLS], BF16, tag="bq")
            bk_sb = const.tile([1, NCOLS], BF16, tag="bk")
            bv_sb = const.tile([1, NCOLS], BF16, tag="bv")
            ones_row = const.tile([1, SB], BF16, tag="ones_row")
            nc.sync.dma_start(bq_sb[:], bq)
            nc.sync.dma_start(bk_sb[:], bk)
            nc.sync.dma_start(bv_sb[:], bv)
            nc.sync.dma_start(ones_row[:], ones_row_d)
        wo_sb = const.tile([128, 2, D], BF16, tag="wo")
        nc.sync.dma_start(wo_sb[:], wo)

        # warm the Exp activation table during the projection phase
        scr = const.tile([1, 8], F32, tag="scr")
        nc.vector.memset(scr[:], 0.0)
        nc.scalar.activation(scr[:], scr[:], mybir.ActivationFunctionType.Exp)

        # --- persistent SBUF tensors
        qt = [[sbig.tile([128, SB], BF16, tag=f"qt{c}_{sb}", name=f"qt{c}_{sb}")
               for sb in range(NSB)] for c in range(2)]
        kth = [[sbig.tile([128, SB], BF16, tag=f"kh{h}_{sb}", name=f"kh{h}_{sb}")
                for sb in range(NSB)] for h in range(HG)]
        for h in range(HG):
            zrows = slice(DK, 128) if h % 2 == 0 else slice(0, DK)
            for sb in range(NSB):
                nc.vector.memset(kth[h][sb][zrows, :], 0.0)
        vaug = [sbig.tile([128, HG * AUGW], BF16, tag=f"va{st}", name=f"va{st}")
                for st in range(NST)]
        for st in range(NST):
            va = vaug[st][:].rearrange("p (h e) -> p h e", h=HG)
            nc.vector.memset(va[:, :, DK:DK + 1], 1.0)
        # normalized O^T head-pair tiles: rows 0-63 head 2c, 64-127 head 2c+1
        otp = [[sbig.tile([128, SB], BF16, tag=f"ot{c}_{j}", name=f"ot{c}_{j}")
                for j in range(NSB)] for c in range(2)]

        # ------------------------------------------------------ unit builders
        def proj_qk_unit(sb, c, kind):
            def run():
                ss = slice(SB * sb, SB * sb + SB)
                ncol = slice(128 * c, 128 * c + 128)
                w_sb = wq_sb if kind == "q" else wk_sb
                ps = pj_ps.tile([128, SB], F32, tag="pj")
                for dc in range(NDC):
                    nc.tensor.matmul(ps[:], w_sb[:, dc, ncol],
                                     xt_tiles[sb][:, dc, :],
                                     start=(dc == 0),
                                     stop=(dc == NDC - 1 and not with_bias))
                if with_bias:
                    b_sb = bq_sb if kind == "q" else bk_sb
                    nc.tensor.matmul(ps[:], b_sb[0:1, ncol], ones_row[0:1, :],
                                     start=False, stop=True)
                t_shuf = rtmp.tile([128, SB], F32, tag="rs")
                nc.vector.stream_shuffle(t_shuf[:], ps[:], SHUF_MASK)
                t_sin = rtmp.tile([128, SB], BF16, tag="rm")
                nc.gpsimd.tensor_mul(t_sin[:], t_shuf[:], sin_sb[:, ss])
                t_cos = rtmp.tile([128, SB], BF16, tag="rc")
                nc.vector.tensor_mul(t_cos[:], ps[:], cos_sb[:, ss])
                if kind == "q":
                    nc.vector.tensor_add(qt[c][sb][:], t_cos[:], t_sin[:])
                else:
                    nc.vector.tensor_add(kth[2 * c][sb][0:DK, :],
                                         t_cos[0:DK, :], t_sin[0:DK, :])
                    nc.vector.tensor_add(kth[2 * c + 1][sb][DK:128, :],
                                         t_cos[DK:128, :], t_sin[DK:128, :])
            return run

        def proj_v_unit(sb, st4):
            def run():
                st = 4 * sb + st4
                ps = pj_ps.tile([128, SB], F32, tag="pj")
                for dc in range(NDC):
                    nc.tensor.matmul(ps[:, 0:NCOLS],
                                     xt_tiles[sb][:, dc, 128 * st4:128 * st4 + 128],
                                     wv_sb[:, dc, :],
                                     start=(dc == 0),
                                     stop=(dc == NDC - 1 and not with_bias))
                if with_bias:
                    nc.tensor.matmul(ps[:, 0:NCOLS], ones_row[0:1, 0:128],
                                     bv_sb[0:1, :], start=False, stop=True)
                va = vaug[st][:].rearrange("p (h e) -> p h e", h=HG)
                nc.vector.tensor_copy(va[:, :, 0:DK],
                                      ps[:, 0:NCOLS].rearrange("p (h k) -> p h k", h=HG))
            return run

        def proj_units(sb):
            us = []
            for c in range(2):
                us.append(proj_qk_unit(sb, c, "q"))
                us.append(proj_qk_unit(sb, c, "k"))
            for st4 in range(4):
                us.append(proj_v_unit(sb, st4))
            return us

        def wo_unit(jb, st4):
            def run():
                st = 4 * jb + st4
                rq = slice(128 * st4, 128 * st4 + 128)
                o_sb = opool.tile([128, 2 * SB], BF16, tag="osb")
                for dc in range(2):
                    cols = slice(SB * dc, SB * dc + SB)
                    ps = pj_ps.tile([128, SB], F32, tag="pj")
                    for cp in range(2):
                        nc.tensor.matmul(ps[:], otp[cp][jb][:, rq],
                                         wo_sb[:, cp, cols],
                                         start=(cp == 0), stop=(cp == 1))
                    nc.vector.tensor_copy(o_sb[:, cols], ps[:])
                    nc.sync.dma_start(out[128 * st:128 * st + 128, cols],
                                      o_sb[:, cols])
            return run

        # ------------------------------------------------------ attention
        pv_tiles = {}
        e_tiles = {}

        def sc_group(j, h, g):
            c = h // 2
            sc = sc_ps.tile([128, GW * SB], F32, tag="sc")
            gd = g - 2 * j  # >= 0 for diagonal groups
            for t in range(GW):
                i = GW * g + t
                nc.tensor.matmul(
                    sc[:, SB * t:SB * t + SB],
                    kth[h][i // 4][:, 128 * (i % 4):128 * (i % 4) + 128],
                    qt[c][j][:],
                    start=True, stop=(gd < 0),
                    skip_group_check=(gd >= 0))
            if gd >= 0:
                # -200 into every (key>query) position of each slab: full
                # columns left of the diagonal sub-block + its triangle
                for t in range(GW):
                    dd = 2 * gd + t
                    w = 128 * (dd + 1)
                    nc.tensor.matmul(sc[:, SB * t:SB * t + w], ident_sb[:],
                                     maskslab_sb[:, dd, 0:w],
                                     start=False, stop=(t == GW - 1),
                                     skip_group_check=True)
            e = epool.tile([128, GW * SB], BF16, tag="e")
            nc.scalar.activation(e[:], sc[:], mybir.ActivationFunctionType.Exp,
                                 scale=SCALE)
            e_tiles[(j, h, g)] = e

        def pv_group(j, h, g, groups_order):
            def run():
                if g == groups_order[0]:
                    pv_tiles[(j, h)] = pv_ps.tile([DK + 1, SB], F32, tag="pv", name=f"pv{j}_{h}")
                pv = pv_tiles[(j, h)]
                e = e_tiles.pop((j, h, g))
                gd = g - 2 * j

                for t in range(GW):
                    i = GW * g + t
                    # queries left of a diagonal key tile are fully masked
                    # (e ~ 1e-11 there) -- skip those columns
                    qs = 128 * (2 * gd + t) if gd >= 0 else 0
                    first = (g == groups_order[0] and t == 0)
                    last = (g == groups_order[-1] and t == GW - 1)
                    if first or last:
                        qs = 0  # start/stop must cover the full pv region
                    lhs = vaug[i][:].rearrange("p (h e) -> p h e", h=HG)[:, h, 0:DK + 1]
                    nc.tensor.matmul(pv[:, qs:SB], lhs,
                                     e[:, SB * t + qs:SB * t + SB],
                                     start=first, stop=last,
                                     skip_group_check=True)
            return run

        def norm_stages(j, h):
            cp, half = divmod(h, 2)
            box = {}

            def s1():  # reciprocal of the denominator row (ones-column of V)
                pv = pv_tiles[(j, h)]
                dn = npool.tile([1, SB], F32, tag="dn")
                nc.vector.tensor_copy(dn[:], pv[DK:DK + 1, :])
                rec = npool.tile([1, SB], F32, tag="rec")
                nc.vector.reciprocal_approx_fast(rec[:], dn[:])
                box["rec"] = rec

            def s2():  # broadcast 1/denom across the 64 dk partitions
                bc = bpool.tile([DK, SB], F32, tag="bc")
                nc.gpsimd.partition_broadcast(bc[:], box["rec"][:], channels=DK)
                box["bc"] = bc

            def s3():  # normalized O^T into the head-pair tile
                pv = pv_tiles.pop((j, h))
                nc.vector.tensor_mul(otp[cp][j][DK * half:DK * half + DK, :],
                                     pv[0:DK, :], box["bc"][:])
            return [s1, s2, s3]

        # ------------------------------------------------------ main schedule
        # heads 0/1 of attn(0) need only the c=0 projections; c=1 units are
        # the first fills inside attn(0)
        proj0 = proj_units(0)
        for u in [proj0[0], proj0[1]] + proj0[4:]:
            u()

        fills = deque()
        norm_q = deque()
        pending_pv = None
        for j in range(NSB):
            if j == 0:
                fills.extend([proj0[2], proj0[3]])
            if j + 1 < NSB:
                fills.append(lambda sb=j + 1: prefetch_xt(sb))
                fills.extend(proj_units(j + 1))
            else:
                for jb in range(NSB - 1):
                    for st4 in range(4):
                        fills.append(wo_unit(jb, st4))
            nfill = len(fills)
            gorder = list(range(2 * j, 2 * (j + 1))) + list(range(2 * j))
            groups = [(h, g) for h in range(HG) for g in gorder]
            ng = len(groups)
            done = 0
            for idx, (h, g) in enumerate(groups):
                sc_group(j, h, g)
                want = (idx + 1) * nfill // ng
                while done < want and fills:
                    fills.popleft()()
                    done += 1
                if pending_pv is not None:
                    pending_pv()
                pending_pv = pv_group(j, h, g, gorder)
                if norm_q:
                    norm_q.popleft()()
                if g == gorder[-1]:
                    norm_q.extend(norm_stages(j, h))
            while fills:
                fills.popleft()()

        pending_pv()
        while norm_q:
            norm_q.popleft()()
        for st4 in range(4):
            wo_unit(NSB - 1, st4)()

    nc.compile()
    return nc


_CACHED_NC = {}


def _get_program(with_bias=False):
    if with_bias not in _CACHED_NC:
        _CACHED_NC[with_bias] = build_program(with_bias=with_bias)
    return _CACHED_NC[with_bias]


# ---------------------------------------------------------------------------
# entry point
# ---------------------------------------------------------------------------

def kernel(x, token_position, Wq, bq, Wk, bk, Wv, bv, Wo, bo, _results=None):
    from concourse.bass_utils import run_bass_kernel_spmd

    in_maps = make_core_inputs(x, token_position, Wq, bq, Wk, bk, Wv, bv, Wo, bo)
    if _results is None:
        with_bias = any(float(np.abs(np.asarray(v)).max()) != 0.0
                        for v in (bq, bk, bv))
        nc = _get_program(with_bias=with_bias)
        res = run_bass_kernel_spmd(nc, in_maps, list(range(N_CORES)))
        _results = [res.results[i]["out"] for i in range(N_CORES)]
    bo = np.asarray(bo, dtype=np.float32)
    out = np.empty((B, S, D), dtype=np.float32)
    for b in range(B):
        acc = np.asarray(_results[HG * b]).astype(np.float32)
        for hg in range(1, HG):
            acc = acc + np.asarray(_results[HG * b + hg]).astype(np.float32)
        out[b] = acc + bo[None, :]
    return out


# revision 18
# speedup vs baseline: 1.0065x; 1.0065x over previous
"""Trainium2 Bass kernel: causal multi-head attention with interleaved RoPE.

Problem shapes (hardcoded): x [2, 2048, 1024], 16 heads of dk=64.
Sharding: 8 cores = 2 batches x 4 head-groups (4 heads each). Each core
computes its head-slice Q/K/V projections, RoPE, causal attention, and a
partial output through its Wo row-slice; the host sums the 4 partials per
batch and adds bo.

Numerics: bf16 operands everywhere (validated ~5e-3 rel err vs the 2e-2
gate), fp32 PSUM accumulation.

Schedule: one flat instruction stream interleaving the three phases so the
PE never idles: projection units of block j+1 and Wo units of block j-1 are
issued as fill between attention score/PV groups of block j (PV lags its
score group by one slot to hide the exp latency). Causal masking accumulates
a -200 mask into the score psum via identity-stationary matmuls; for j>0
the score/exp/PV work on diagonal groups is trimmed to the columns right of
each key tile (the fully-masked left columns are skipped outright, so only
the 128-wide diagonal triangle needs masking). Softmax normalization runs
entirely off the PE: denominator row staged to SBUF (custom-DVE ops cannot
read PSUM on hw), reciprocal_approx_fast (DVE) -> partition_broadcast
(gpsimd) -> mul (DVE). Wo packs head pairs on the contraction dim (128 rows
fully used). DMA: few large blocked transfers split across both hwdge
queues (sync + Activation) so the first projection starts ~11us in; outputs
stream out per 128-row block as Wo psums complete.

RoPE trick: attention scores are invariant to any permutation of the dk
axis applied to both Q and K, so the Wq/Wk columns are permuted on the host
into a "quadrant half-split" layout where each rotation pair partner sits
exactly 16 partitions away inside the same 32-partition quadrant. The DVE
stream_shuffle (a per-quadrant 32-way permute) then produces the swapped
operand, and RoPE becomes: rot = q * cosT + shuffle(q) * sinT with
host-precomputed tables (sinT carries the sign).
"""

from collections import deque
from contextlib import ExitStack

import numpy as np
import ml_dtypes

import concourse.bass as bass
import concourse.mybir as mybir
import concourse.tile as tile

B, S, D, H = 2, 2048, 1024, 16
DK = D // H  # 64
HG = 4  # heads per core
NCOLS = HG * DK  # 256 columns of the projection per core
THETA = 10000.0
SCALE = 1.0 / float(np.sqrt(DK))
N_CORES = 8

SB = 512            # sq block width
NSB = S // SB       # 4
NST = S // 128      # 16 key tiles / V tiles
NDC = D // 128      # 8 contraction chunks
GW = 2              # key tiles per score-psum group
AUGW = 72           # V head stride (65 used), 72*2B = 144 = 9*16B aligned

F32 = mybir.dt.float32
F32R = mybir.dt.float32r
BF16 = mybir.dt.bfloat16
BF = ml_dtypes.bfloat16
MASKVAL = -200.0


# ---------------------------------------------------------------------------
# host-side prep
# ---------------------------------------------------------------------------

def _rope_perm():
    """Within-head column permutation pi: new row r -> original dk index."""
    perm = np.empty(DK, dtype=np.int64)
    for r in range(DK):
        q, m = divmod(r, 32)
        if m < 16:
            perm[r] = 2 * (16 * q + m)
        else:
            perm[r] = 2 * (16 * q + m - 16) + 1
    return perm


_PERM = _rope_perm()
SHUF_MASK = list(range(16, 32)) + list(range(16))  # swap 16-halves per quadrant


def _rope_tables(pos):
    """cosT/sinT [128, S] fp32 for the permuted layout. pos: [S] int."""
    inv_freq = (np.float32(THETA) ** (-(np.arange(0, DK, 2, dtype=np.float32) / np.float32(DK))))  # [32]
    ang = pos.astype(np.float32)[:, None] * inv_freq[None, :]  # [S, 32]
    cos = np.cos(ang)
    sin = np.sin(ang)
    cosT = np.empty((128, S), dtype=np.float32)
    sinT = np.empty((128, S), dtype=np.float32)
    for p in range(128):
        r = p % DK
        q, m = divmod(r, 32)
        if m < 16:
            i = 16 * q + m
            sgn = -1.0
        else:
            i = 16 * q + m - 16
            sgn = 1.0
        cosT[p] = cos[:, i]
        sinT[p] = np.float32(sgn) * sin[:, i]
    return cosT, sinT


def bf16(a):
    return np.ascontiguousarray(np.asarray(a, dtype=np.float32).astype(BF))


def make_core_inputs(x, token_position, Wq, bq, Wk, bk, Wv, bv, Wo, bo):
    """Build the 8 per-core input maps."""
    x = np.asarray(x, dtype=np.float32)
    token_position = np.asarray(token_position)
    Wq, Wk, Wv, Wo = (np.asarray(w, dtype=np.float32) for w in (Wq, Wk, Wv, Wo))
    bq, bk, bv = (np.asarray(b_, dtype=np.float32) for b_ in (bq, bk, bv))

    # mask slabs for the 4 diagonal key-tile offsets d: key 128d+p masks
    # query q (tile-relative) iff 128d+p > q; -200 added into the score psum
    pp = np.arange(128)[:, None]
    qq = np.arange(SB)[None, :]
    maskslab = bf16(np.stack(
        [np.where(128 * dd + pp > qq, np.float32(MASKVAL), np.float32(0.0))
         for dd in range(4)], axis=1))  # [128, 4, 512]
    ident = bf16(np.eye(128, dtype=np.float32))
    onesr = np.ones((1, DK), dtype=np.float32)

    in_maps = []
    tables = {}
    for c in range(N_CORES):
        b, hg = divmod(c, HG)
        heads = range(HG * hg, HG * hg + HG)
        cols_qk = np.concatenate([DK * h + _PERM for h in heads])
        cols_v = np.arange(NCOLS * hg, NCOLS * hg + NCOLS)
        if b not in tables:
            tables[b] = _rope_tables(np.asarray(token_position[b]))
        cosT, sinT = tables[b]
        # wo rows packed as head pairs: [:, cpair, :] rows 0-63 head 2c, 64-127 head 2c+1
        wo_dev = np.stack([Wo[cols_v[128 * cp:128 * cp + 128], :] for cp in range(2)],
                          axis=1)  # [128, 2, 1024]
        def blk_w(w):  # [1024, 256] -> [128, 8, 256] (dc-blocked, partition-major)
            return bf16(np.ascontiguousarray(w.reshape(NDC, 128, NCOLS).transpose(1, 0, 2)))
        xT_blk = bf16(np.ascontiguousarray(
            x[b].T.reshape(NDC, 128, NSB, SB).transpose(2, 1, 0, 3)))  # [4, 128, 8, 512]
        in_maps.append({
            "xT": xT_blk,                             # [4, 128, 8, 512]
            "wq": blk_w(Wq[:, cols_qk]),              # [128, 8, 256]
            "wk": blk_w(Wk[:, cols_qk]),
            "wv": blk_w(Wv[:, cols_v]),
            "wo": bf16(wo_dev),                       # [128, 2, 1024]
            "bq": bf16(bq[cols_qk][None, :]),         # [1, 256]
            "bk": bf16(bk[cols_qk][None, :]),
            "bv": bf16(bv[cols_v][None, :]),
            "ones_row": bf16(np.ones((1, SB), np.float32)),
            "onesr": onesr,                           # [1, 64] fp32 (f32r param)
            "ident": ident,                           # [128, 128]
            "maskslab": maskslab,                     # [128, 4, 512]
            "cosT": bf16(cosT),
            "sinT": bf16(sinT),
        })
    return in_maps


# ---------------------------------------------------------------------------
# device program
# ---------------------------------------------------------------------------

def build_program(with_bias=False):
    from concourse import bacc, library_config
    nc = bacc.Bacc("TRN2", debug=False)

    xT = nc.declare_dram_parameter("xT", [NSB, 128, NDC, SB], BF16, isOutput=False).ap()
    wq = nc.declare_dram_parameter("wq", [128, NDC, NCOLS], BF16, isOutput=False).ap()
    wk = nc.declare_dram_parameter("wk", [128, NDC, NCOLS], BF16, isOutput=False).ap()
    wv = nc.declare_dram_parameter("wv", [128, NDC, NCOLS], BF16, isOutput=False).ap()
    wo = nc.declare_dram_parameter("wo", [128, 2, D], BF16, isOutput=False).ap()
    bq = nc.declare_dram_parameter("bq", [1, NCOLS], BF16, isOutput=False).ap()
    bk = nc.declare_dram_parameter("bk", [1, NCOLS], BF16, isOutput=False).ap()
    bv = nc.declare_dram_parameter("bv", [1, NCOLS], BF16, isOutput=False).ap()
    ones_row_d = nc.declare_dram_parameter("ones_row", [1, SB], BF16, isOutput=False).ap()
    onesr_d = nc.declare_dram_parameter("onesr", [1, DK], F32R, isOutput=False).ap()
    ident_d = nc.declare_dram_parameter("ident", [128, 128], BF16, isOutput=False).ap()
    maskslab_d = nc.declare_dram_parameter("maskslab", [128, 4, SB], BF16, isOutput=False).ap()
    cosT = nc.declare_dram_parameter("cosT", [128, S], BF16, isOutput=False).ap()
    sinT = nc.declare_dram_parameter("sinT", [128, S], BF16, isOutput=False).ap()
    out = nc.declare_dram_parameter("out", [S, D], BF16, isOutput=True).ap()

    with tile.TileContext(nc) as tc, ExitStack() as ctx:
        nc.gpsimd.load_library(library_config.proxy)
        const = ctx.enter_context(tc.tile_pool(name="const", bufs=1))
        sbig = ctx.enter_context(tc.tile_pool(name="sbig", bufs=1))
        xts = ctx.enter_context(tc.tile_pool(name="xts", bufs=2))
        rtmp = ctx.enter_context(tc.tile_pool(name="rtmp", bufs=2))
        epool = ctx.enter_context(tc.tile_pool(name="epool", bufs=3))
        npool = ctx.enter_context(tc.tile_pool(name="npool", bufs=2))
        bpool = ctx.enter_context(tc.tile_pool(name="bpool", bufs=2))
        opool = ctx.enter_context(tc.tile_pool(name="opool", bufs=2))
        pj_ps = ctx.enter_context(tc.tile_pool(name="pj_ps", bufs=2, space="PSUM"))
        sc_ps = ctx.enter_context(tc.tile_pool(name="sc_ps", bufs=2, space="PSUM"))
        pv_ps = ctx.enter_context(tc.tile_pool(name="pv_ps", bufs=2, space="PSUM"))

        # --- DMA prelude: few large transfers, first projection's operands first
        wqa = const.tile([128, 4, NCOLS], BF16, tag="wqa")
        wqb = const.tile([128, 4, NCOLS], BF16, tag="wqb")
        wk_sb = const.tile([128, NDC, NCOLS], BF16, tag="wk")
        wv_sb = const.tile([128, NDC, NCOLS], BF16, tag="wv")
        xt_tiles = {}

        def prefetch_xt(sb):
            t = xts.tile([128, NDC, SB], BF16, tag="xt", name=f"xt{sb}")
            nc.sync.dma_start(t[:], xT[sb])
            xt_tiles[sb] = t

        # first block as separate half-tiles: per-tile dependencies let the
        # dc 0-3 matmuls start before the dc 4-7 bytes land; wk rides the
        # second (Activation) hwdge queue meanwhile
        xt0a = xts.tile([128, 4, SB], BF16, tag="xta", name="xt0a")
        xt0b = xts.tile([128, 4, SB], BF16, tag="xtb", name="xt0b")
        nc.sync.dma_start(wqa[:], wq[:, 0:4, :])
        nc.sync.dma_start(xt0a[:], xT[0, :, 0:4, :])
        cos_sb = const.tile([128, S], BF16, tag="cos")
        sin_sb = const.tile([128, S], BF16, tag="sin")
        nc.scalar.dma_start(wk_sb[:], wk)
        nc.scalar.dma_start(cos_sb[:, 0:SB], cosT[:, 0:SB])
        nc.scalar.dma_start(sin_sb[:, 0:SB], sinT[:, 0:SB])
        nc.sync.dma_start(wqb[:], wq[:, 4:, :])
        nc.sync.dma_start(xt0b[:], xT[0, :, 4:, :])
        xt_tiles[0] = None  # sb=0 uses xt0a/xt0b via xt_slice
        ident_sb = const.tile([128, 128], BF16, tag="ident")
        nc.scalar.dma_start(ident_sb[:], ident_d)
        maskslab_sb = const.tile([128, 4, SB], BF16, tag="maskslab")
        nc.scalar.dma_start(maskslab_sb[:], maskslab_d)
        nc.scalar.dma_start(wv_sb[:], wv)
        nc.scalar.dma_start(cos_sb[:, SB:], cosT[:, SB:])
        nc.scalar.dma_start(sin_sb[:, SB:], sinT[:, SB:])
        onesr_sb = const.tile([1, DK], F32R, tag="onesr")
        nc.sync.dma_start(onesr_sb[:], onesr_d)
        if with_bias:
            bq_sb = const.tile([1, NCOLS], BF16, tag="bq")
            bk_sb = const.tile([1, NCOLS], BF16, tag="bk")
            bv_sb = const.tile([1, NCOLS], BF16, tag="bv")
            ones_row = const.tile([1, SB], BF16, tag="ones_row")
            nc.sync.dma_start(bq_sb[:], bq)
            nc.sync.dma_start(bk_sb[:], bk)
            nc.sync.dma_start(bv_sb[:], bv)
            nc.sync.dma_start(ones_row[:], ones_row_d)
        wo_sb = const.tile([128, 2, D], BF16, tag="wo")
        nc.sync.dma_start(wo_sb[:], wo)

        def xt_slice(sb, dc):
            if sb == 0:
                return (xt0a if dc < 4 else xt0b)[:, dc % 4, :]
            return xt_tiles[sb][:, dc, :]

        def wq_slice(dc, ncol):
            return (wqa if dc < 4 else wqb)[:, dc % 4, ncol]

        # warm the Exp activation table during the projection phase (reads
        # uninitialized scratch -- output is never consumed)
        scr = const.tile([1, 8], F32, tag="scr")
        nc.scalar.activation(scr[:], scr[:], mybir.ActivationFunctionType.Exp)

        # --- persistent SBUF tensors
        qt = [[sbig.tile([128, SB], BF16, tag=f"qt{c}_{sb}", name=f"qt{c}_{sb}")
               for sb in range(NSB)] for c in range(2)]
        kth = [[sbig.tile([128, SB], BF16, tag=f"kh{h}_{sb}", name=f"kh{h}_{sb}")
                for sb in range(NSB)] for h in range(HG)]
        for sb in range(NSB):
            for h in range(HG):
                zrows = slice(DK, 128) if h % 2 == 0 else slice(0, DK)
                nc.vector.memset(kth[h][sb][zrows, :], 0.0)
        vaug = [sbig.tile([128, HG * AUGW], BF16, tag=f"va{st}", name=f"va{st}")
                for st in range(NST)]
        for st in range(NST):
            va = vaug[st][:].rearrange("p (h e) -> p h e", h=HG)
            nc.gpsimd.memset(va[:, :, DK:DK + 1], 1.0)
        # normalized O^T head-pair tiles: rows 0-63 head 2c, 64-127 head 2c+1
        otp = [[sbig.tile([128, SB], BF16, tag=f"ot{c}_{j}", name=f"ot{c}_{j}")
                for j in range(NSB)] for c in range(2)]

        # ------------------------------------------------------ unit builders
        def proj_qk_unit(sb, c, kind):
            def run():
                ss = slice(SB * sb, SB * sb + SB)
                ncol = slice(128 * c, 128 * c + 128)
                ps = pj_ps.tile([128, SB], F32, tag="pj")
                for dc in range(NDC):
                    w_ap = (wq_slice(dc, ncol) if kind == "q"
                            else wk_sb[:, dc, ncol])
                    nc.tensor.matmul(ps[:], w_ap, xt_slice(sb, dc),
                                     start=(dc == 0),
                                     stop=(dc == NDC - 1 and not with_bias))
                if with_bias:
                    b_sb = bq_sb if kind == "q" else bk_sb
                    nc.tensor.matmul(ps[:], b_sb[0:1, ncol], ones_row[0:1, :],
                                     start=False, stop=True)
                t_shuf = rtmp.tile([128, SB], F32, tag="rs")
                nc.vector.stream_shuffle(t_shuf[:], ps[:], SHUF_MASK)
                t_sin = rtmp.tile([128, SB], BF16, tag="rm")
                nc.gpsimd.tensor_mul(t_sin[:], t_shuf[:], sin_sb[:, ss])
                t_cos = rtmp.tile([128, SB], BF16, tag="rc")
                nc.vector.tensor_mul(t_cos[:], ps[:], cos_sb[:, ss])
                if kind == "q":
                    nc.vector.tensor_add(qt[c][sb][:], t_cos[:], t_sin[:])
                else:
                    nc.vector.tensor_add(kth[2 * c][sb][0:DK, :],
                                         t_cos[0:DK, :], t_sin[0:DK, :])
                    nc.vector.tensor_add(kth[2 * c + 1][sb][DK:128, :],
                                         t_cos[DK:128, :], t_sin[DK:128, :])
            return run

        def proj_v_unit(sb, st4):
            def run():
                st = 4 * sb + st4
                ps = pj_ps.tile([128, SB], F32, tag="pj")
                for dc in range(NDC):
                    xs = xt_slice(sb, dc)
                    nc.tensor.matmul(ps[:, 0:NCOLS],
                                     xs[:, 128 * st4:128 * st4 + 128],
                                     wv_sb[:, dc, :],
                                     start=(dc == 0),
                                     stop=(dc == NDC - 1 and not with_bias))
                if with_bias:
                    nc.tensor.matmul(ps[:, 0:NCOLS], ones_row[0:1, 0:128],
                                     bv_sb[0:1, :], start=False, stop=True)
                va = vaug[st][:].rearrange("p (h e) -> p h e", h=HG)
                nc.vector.tensor_copy(va[:, :, 0:DK],
                                      ps[:, 0:NCOLS].rearrange("p (h k) -> p h k", h=HG))
            return run

        def proj_units(sb):
            us = []
            for c in range(2):
                us.append(proj_qk_unit(sb, c, "q"))
                us.append(proj_qk_unit(sb, c, "k"))
            for st4 in range(4):
                us.append(proj_v_unit(sb, st4))
            return us

        def wo_unit(jb, st4):
            def run():
                st = 4 * jb + st4
                rq = slice(128 * st4, 128 * st4 + 128)
                o_sb = opool.tile([128, 2 * SB], BF16, tag="osb")
                for dc in range(2):
                    cols = slice(SB * dc, SB * dc + SB)
                    ps = pj_ps.tile([128, SB], F32, tag="pj")
                    for cp in range(2):
                        nc.tensor.matmul(ps[:], otp[cp][jb][:, rq],
                                         wo_sb[:, cp, cols],
                                         start=(cp == 0), stop=(cp == 1))
                    nc.vector.tensor_copy(o_sb[:, cols], ps[:])
                    nc.sync.dma_start(out[128 * st:128 * st + 128, cols],
                                      o_sb[:, cols])
            return run

        # ------------------------------------------------------ attention
        pv_tiles = {}
        e_tiles = {}

        def sc_group(j, h, g):
            c = h // 2
            sc = sc_ps.tile([128, GW * SB], F32, tag="sc")
            gd = g - 2 * j  # >= 0 for diagonal groups
            for t in range(GW):
                i = GW * g + t
                nc.tensor.matmul(
                    sc[:, SB * t:SB * t + SB],
                    kth[h][i // 4][:, 128 * (i % 4):128 * (i % 4) + 128],
                    qt[c][j][:],
                    start=True, stop=(gd < 0),
                    skip_group_check=(gd >= 0))
            if gd >= 0:
                # -200 into every (key>query) position of each slab: full
                # columns left of the diagonal sub-block + its triangle
                for t in range(GW):
                    dd = 2 * gd + t
                    w = 128 * (dd + 1)
                    nc.tensor.matmul(sc[:, SB * t:SB * t + w], ident_sb[:],
                                     maskslab_sb[:, dd, 0:w],
                                     start=False, stop=(t == GW - 1),
                                     skip_group_check=True)
            e = epool.tile([128, GW * SB], BF16, tag="e")
            nc.scalar.activation(e[:], sc[:], mybir.ActivationFunctionType.Exp,
                                 scale=SCALE)
            e_tiles[(j, h, g)] = e

        def pv_group(j, h, g, groups_order):
            def run():
                if g == groups_order[0]:
                    pv_tiles[(j, h)] = pv_ps.tile([DK + 1, SB], F32, tag="pv", name=f"pv{j}_{h}")
                pv = pv_tiles[(j, h)]
                e = e_tiles.pop((j, h, g))
                gd = g - 2 * j

                for t in range(GW):
                    i = GW * g + t
                    # queries left of a diagonal key tile are fully masked
                    # (e ~ 1e-11 there) -- skip those columns
                    qs = 128 * (2 * gd + t) if gd >= 0 else 0
                    first = (g == groups_order[0] and t == 0)
                    last = (g == groups_order[-1] and t == GW - 1)
                    if first or last:
                        qs = 0  # start/stop must cover the full pv region
                    lhs = vaug[i][:].rearrange("p (h e) -> p h e", h=HG)[:, h, 0:DK + 1]
                    nc.tensor.matmul(pv[:, qs:SB], lhs,
                                     e[:, SB * t + qs:SB * t + SB],
                                     start=first, stop=last,
                                     skip_group_check=True)
            return run

        def norm_stages(j, h):
            cp, half = divmod(h, 2)
            box = {}

            def s1():  # reciprocal of the denominator row (ones-column of V)
                pv = pv_tiles[(j, h)]
                dn = npool.tile([1, SB], F32, tag="dn")
                nc.vector.tensor_copy(dn[:], pv[DK:DK + 1, :])
                rec = npool.tile([1, SB], F32, tag="rec")
                nc.vector.reciprocal_approx_fast(rec[:], dn[:])
                box["rec"] = rec

            def s2():  # broadcast 1/denom across the 64 dk partitions
                bc = bpool.tile([DK, SB], F32, tag="bc")
                nc.gpsimd.partition_broadcast(bc[:], box["rec"][:], channels=DK)
                box["bc"] = bc

            def s3():  # normalized O^T into the head-pair tile
                pv = pv_tiles.pop((j, h))
                nc.vector.tensor_mul(otp[cp][j][DK * half:DK * half + DK, :],
                                     pv[0:DK, :], box["bc"][:])
            return [s1, s2, s3]

        # ------------------------------------------------------ main schedule
        # heads 0/1 of attn(0) need only the c=0 projections; c=1 units are
        # the first fills inside attn(0)
        proj0 = proj_units(0)
        for u in [proj0[0], proj0[1]] + proj0[4:]:
            u()

        fills = deque()
        norm_q = deque()
        pending_pv = None
        for j in range(NSB):
            if j == 0:
                fills.extend([proj0[2], proj0[3]])
            if j + 1 < NSB:
                fills.append(lambda sb=j + 1: prefetch_xt(sb))
                fills.extend(proj_units(j + 1))
            else:
                for jb in range(NSB - 1):
                    for st4 in range(4):
                        fills.append(wo_unit(jb, st4))
            nfill = len(fills)
            gorder = list(range(2 * j, 2 * (j + 1))) + list(range(2 * j))
            groups = [(h, g) for h in range(HG) for g in gorder]
            ng = len(groups)
            done = 0
            for idx, (h, g) in enumerate(groups):
                sc_group(j, h, g)
                want = (idx + 1) * nfill // ng
                while done < want and fills:
                    fills.popleft()()
                    done += 1
                if pending_pv is not None:
                    pending_pv()
                pending_pv = pv_group(j, h, g, gorder)
                if norm_q:
                    norm_q.popleft()()
                if g == gorder[-1]:
                    norm_q.extend(norm_stages(j, h))
            while fills:
                fills.popleft()()

        pending_pv()
        while norm_q:
            norm_q.popleft()()
        for st4 in range(4):
            wo_unit(NSB - 1, st4)()

    nc.compile()
    return nc


_CACHED_NC = {}


def _get_program(with_bias=False):
    if with_bias not in _CACHED_NC:
        _CACHED_NC[with_bias] = build_program(with_bias=with_bias)
    return _CACHED_NC[with_bias]


# ---------------------------------------------------------------------------
# entry point
# ---------------------------------------------------------------------------

def kernel(x, token_position, Wq, bq, Wk, bk, Wv, bv, Wo, bo, _results=None):
    from concourse.bass_utils import run_bass_kernel_spmd

    in_maps = make_core_inputs(x, token_position, Wq, bq, Wk, bk, Wv, bv, Wo, bo)
    if _results is None:
        with_bias = any(float(np.abs(np.asarray(v)).max()) != 0.0
                        for v in (bq, bk, bv))
        nc = _get_program(with_bias=with_bias)
        res = run_bass_kernel_spmd(nc, in_maps, list(range(N_CORES)))
        _results = [res.results[i]["out"] for i in range(N_CORES)]
    bo = np.asarray(bo, dtype=np.float32)
    out = np.empty((B, S, D), dtype=np.float32)
    for b in range(B):
        acc = np.asarray(_results[HG * b]).astype(np.float32)
        for hg in range(1, HG):
            acc = acc + np.asarray(_results[HG * b + hg]).astype(np.float32)
        out[b] = acc + bo[None, :]
    return out
